# revision 13
# baseline (speedup 1.0000x reference)
"""Trainium2 Bass kernel for nn_ConditionalFeedForward (MoE routed SwiGLU FFN).

Strategy (expert-parallel, routed):
  - The reference computes every expert for every token, then gathers the
    TOP_K=2 routed experts.  Only the routed (token, expert) pairs are
    needed, so on the host we bucket tokens by expert (deduplicating
    tokens that pick the same expert twice), pad each bucket to a fixed
    capacity C, and give expert e's bucket to NeuronCore e (E=8 experts,
    8 cores).
  - Each core computes  y = (silu(xg @ w1e.T) * (xg @ w3e.T)) @ w2e.T
    for its C routed tokens with fp32r (FP22-truncated, full-rate)
    matmuls, all intermediates resident in SBUF.
  - The host scatters each core's rows back into the (T, TOP_K, D) output.

Device dataflow per core (all tokens of one expert):
  phase 1:  h1T/h3T tiles [h=128, c<=512] accumulate over d in PSUM;
            silu+mul drain into a resident SBUF tensor a[H, C].
  phase 2:  y[c=128, dd<=512] accumulates over all 32 h-tiles in PSUM
            (lhsT = a-tile, rhs = w2T tile), then drains to DRAM.
"""

import numpy as np

# Environment paths for the concourse/bass stack (present on the runner
# image; PYTHONPATH normally already includes them).
import sys

for _p in ("/opt/trn_rl_repo", "/root/.axon_site/_ro/trn_rl_repo"):
    if _p not in sys.path:
        sys.path.append(_p)

T = 4096
E = 8
D = 1024
H = 4096
TOP_K = 2
P = 128  # SBUF partitions

_PROG_CACHE: dict = {}
LAST_RUN = None  # BassKernelResults of the most recent device run (for test.py)


def _build_program(C: int, Dm: int, Hm: int):
    """Emit the per-core Bass/Tile program for capacity-C routed tokens."""
    import concourse.bass as bass  # noqa: F401
    import concourse.mybir as mybir
    from concourse import bacc
    from concourse.tile import TileContext

    f32 = mybir.dt.float32
    f32r = mybir.dt.float32r
    SIG = mybir.ActivationFunctionType.Sigmoid

    KD = Dm // P          # d-tiles (contraction of phase 1)
    NH = Hm // P          # h-tiles
    CC = C // 512         # 512-wide token chunks
    CT = C // P           # 128-wide token tiles (phase-2 output partitions)
    DDH = Dm // 512       # 512-wide output-dim chunks

    nc = bacc.Bacc("TRN2", target_bir_lowering=False)

    xgt = nc.dram_tensor("xgt", [Dm, C], f32r, kind="ExternalInput")
    w1t = nc.dram_tensor("w1t", [Dm, Hm], f32r, kind="ExternalInput")
    w3t = nc.dram_tensor("w3t", [Dm, Hm], f32r, kind="ExternalInput")
    w2t = nc.dram_tensor("w2t", [Hm, Dm], f32r, kind="ExternalInput")
    y = nc.dram_tensor("y", [C, Dm], f32, kind="ExternalOutput")

    xgt_r = xgt[:].rearrange("(do di) c -> di do c", di=P)   # [128, KD, C]
    w1t_r = w1t[:].rearrange("(do di) h -> di do h", di=P)   # [128, KD, H]
    w3t_r = w3t[:].rearrange("(do di) h -> di do h", di=P)

    with TileContext(nc) as tc:
        with (
            tc.tile_pool(name="xg", bufs=1) as xg_pool,
            tc.tile_pool(name="abuf", bufs=1) as a_pool,
            tc.tile_pool(name="w13", bufs=2) as w13_pool,
            tc.tile_pool(name="w2", bufs=4) as w2_pool,
            tc.tile_pool(name="tmp", bufs=3) as tmp_pool,
            tc.tile_pool(name="ydrain", bufs=8) as ydrain_pool,
        ):
            # h=0 weight tiles are the first matmul's dependency — issue
            # their DMAs before the big xg load so the queues serve them first
            w1_sb0 = w13_pool.tile([P, KD, P], f32r, tag="w1")
            nc.sync.dma_start(out=w1_sb0, in_=w1t_r[:, :, 0:P])
            w3_sb0 = w13_pool.tile([P, KD, P], f32r, tag="w3")
            nc.sync.dma_start(out=w3_sb0, in_=w3t_r[:, :, 0:P])

            xg_sb = xg_pool.tile([P, KD, C], f32r)
            # d=0 first: the first accumulation's rhs; remaining d-tiles
            # stream in while the first matmuls run
            for d in range(KD):
                nc.sync.dma_start(out=xg_sb[:, d, :], in_=xgt_r[:, d, :])
            a_sb = a_pool.tile([P, NH, C], f32r)

            # ---- phase 1: a[h, c] = silu(w1.T x) * (w3.T x) ----
            with tc.tile_pool(name="ps1", bufs=8, space="PSUM") as ps1:
                for h in range(NH):
                    hs = slice(h * P, (h + 1) * P)
                    if h == 0:
                        w1_sb, w3_sb = w1_sb0, w3_sb0
                    else:
                        w1_sb = w13_pool.tile([P, KD, P], f32r, tag="w1")
                        nc.sync.dma_start(out=w1_sb, in_=w1t_r[:, :, hs])
                        w3_sb = w13_pool.tile([P, KD, P], f32r, tag="w3")
                        nc.sync.dma_start(out=w3_sb, in_=w3t_r[:, :, hs])

                    h1_ps = [
                        ps1.tile([P, 512], f32, tag="ps", name=f"h1_{h}_{i}")
                        for i in range(CC)
                    ]
                    h3_ps = [
                        ps1.tile([P, 512], f32, tag="ps", name=f"h3_{h}_{i}")
                        for i in range(CC)
                    ]
                    for d in range(KD):
                        for cc in range(CC):
                            cs = slice(cc * 512, (cc + 1) * 512)
                            nc.tensor.matmul(
                                h1_ps[cc],
                                w1_sb[:, d, :],
                                xg_sb[:, d, cs],
                                start=(d == 0),
                                stop=(d == KD - 1),
                            )
                    for d in range(KD):
                        for cc in range(CC):
                            cs = slice(cc * 512, (cc + 1) * 512)
                            nc.tensor.matmul(
                                h3_ps[cc],
                                w3_sb[:, d, :],
                                xg_sb[:, d, cs],
                                start=(d == 0),
                                stop=(d == KD - 1),
                            )
                    for cc in range(CC):
                        cs = slice(cc * 512, (cc + 1) * 512)
                        s_sb = tmp_pool.tile([P, 512], f32, tag="s")
                        nc.scalar.activation(s_sb, h1_ps[cc], SIG)
                        nc.vector.tensor_mul(out=s_sb, in0=s_sb, in1=h1_ps[cc])
                        nc.vector.tensor_mul(
                            out=a_sb[:, h, cs], in0=s_sb, in1=h3_ps[cc]
                        )

            # ---- phase 2: y[c, dd] = sum_h a[h, c].T @ w2T[h, dd] ----
            # groups of <=8 token tiles so PSUM holds every accumulator
            groups = []
            for ddh in range(DDH):
                for c0 in range(0, CT, 8):
                    groups.append((ddh, list(range(c0, min(c0 + 8, CT)))))
            PREFETCH_H = 2  # next-group w2 tiles issued before this group's drains

            def w2_load(ddh, h):
                dds = slice(ddh * 512, (ddh + 1) * 512)
                w2_sb = w2_pool.tile([P, 512], f32r, tag="w2", name=f"w2_{ddh}_{h}")
                nc.sync.dma_start(out=w2_sb, in_=w2t[h * P : (h + 1) * P, dds])
                return w2_sb

            with tc.tile_pool(name="ps2", bufs=8, space="PSUM") as ps2:
                prefetched: dict = {}
                for gi, (ddh, cts) in enumerate(groups):
                    dds = slice(ddh * 512, (ddh + 1) * 512)
                    y_ps = {
                        c: ps2.tile([P, 512], f32, tag="y", name=f"y_{ddh}_{c}")
                        for c in cts
                    }
                    for h in range(NH):
                        w2_sb = prefetched.pop((gi, h), None)
                        if w2_sb is None:
                            w2_sb = w2_load(ddh, h)
                        for c in cts:
                            nc.tensor.matmul(
                                y_ps[c],
                                a_sb[:, h, c * P : (c + 1) * P],
                                w2_sb[:],
                                start=(h == 0),
                                stop=(h == NH - 1),
                            )
                    # keep the sync ring free for the next group's weights:
                    # issue those loads first, and push the drains through the
                    # scalar HWDGE ring instead of sync
                    if gi + 1 < len(groups):
                        nddh, _ = groups[gi + 1]
                        for h in range(PREFETCH_H):
                            prefetched[(gi + 1, h)] = w2_load(nddh, h)
                    for c in cts:
                        y_sb = ydrain_pool.tile([P, 512], f32, tag="ysb")
                        nc.vector.tensor_copy(out=y_sb, in_=y_ps[c])
                        nc.scalar.dma_start(
                            out=y[c * P : (c + 1) * P, dds], in_=y_sb
                        )
    nc.compile()  # bacc passes: split multi-waits, alloc regs, fuse nops
    return nc


def _get_program(C: int, Dm: int, Hm: int):
    key = (C, Dm, Hm)
    if key not in _PROG_CACHE:
        _PROG_CACHE[key] = _build_program(C, Dm, Hm)
    return _PROG_CACHE[key]


def kernel(x, expert_indices, w1, w2, w3):
    global LAST_RUN
    from concourse.bass_utils import run_bass_kernel_spmd

    x = np.ascontiguousarray(np.asarray(x, dtype=np.float32))
    idx = np.asarray(expert_indices)
    w1 = np.asarray(w1, dtype=np.float32)
    w2 = np.asarray(w2, dtype=np.float32)
    w3 = np.asarray(w3, dtype=np.float32)

    Tn, Kn = idx.shape
    Dm = x.shape[1]
    En, Hm, _ = w1.shape
    assert En == 8, f"kernel is hardcoded for 8 experts on 8 cores, got {En}"
    idx64 = idx.astype(np.int64)

    # Host routing: unique token list per expert.
    toks = [np.nonzero((idx64 == e).any(axis=1))[0] for e in range(En)]
    maxc = max(len(t) for t in toks)
    C = max(1024, -(-maxc // 512) * 512)

    nc = _get_program(C, Dm, Hm)

    in_maps = []
    for e in range(En):
        te = toks[e]
        xg = np.zeros((C, Dm), np.float32)
        xg[: len(te)] = x[te]
        in_maps.append(
            {
                "xgt": np.ascontiguousarray(xg.T),
                "w1t": np.ascontiguousarray(w1[e].T),
                "w3t": np.ascontiguousarray(w3[e].T),
                "w2t": np.ascontiguousarray(w2[e].T),
            }
        )

    LAST_RUN = run_bass_kernel_spmd(nc, in_maps, list(range(En)))
    res = LAST_RUN.results

    out = np.empty((Tn, Kn, Dm), np.float32)
    for e in range(En):
        t_arr, k_arr = np.nonzero(idx64 == e)
        pos = np.searchsorted(toks[e], t_arr)
        out[t_arr, k_arr] = res[e]["y"][pos]
    return out
